# revision 19
# baseline (speedup 1.0000x reference)
"""Trainium2 Bass kernel for the Dial2vec contrastive loss (nn_Dial2vec).

Math: the dense reference collapses (see v1 notes) to, per sequence,
    Q_T[t] = sum_{turn_l=t} a_l h_l ; R_T[t] likewise with b        [16, H]
    gam_l  = a_l h_l.(Band R_T)[turn_l] + b_l h_l.(Band Q_T)[turn_l]
    qs = sum a_l h_l ; qc = sum a_l gam_l h_l ; rs/rc with b
followed by a host-side O(B*H) cosine / log-softmax reduction.  Cosine
similarity is scale-invariant, so mask-count denominators and a global
2^-6 scale on gam cancel.

Device dataflow (v2) — everything runs as small PE matmuls in fp8e4
(end-to-end quantization error ~2e-4, vs the 2e-2 gate):

  G-pass   Gt[hb] = h_blk(ci,hb)^T @ ABX_ci        [128h, 32] x6, psum-acc
           (ABX = band-smeared one-hot masks, so Gt = [Band R_T; Band Q_T]^T
           directly — no separate band smear and no transposes)
  U-pass   U_ci  = hT_blk(hb,ci)^T @ Gt[hb]        [128tok, 32], psum-acc
  gam      one fused DVE tensor_tensor_reduce per chunk:
           gam = rowsum(U ∘ AB2) * 2^-6            [128, 1]
  D-pass   out = [a, b, a*gam, b*gam]^T @ h_chunk  [4, H] rows =
           [qs, rs, qc*2^-6, rc*2^-6], two sequences column-tiled per wave.

h is shipped twice (row-major for G/D, transposed for U) because the PE
contracts over partitions only; fp8 keeps the total input at ~6.3 MB/core.
Host does index-only mask prep + the final 40-vector reduction.
"""

import os

import numpy as np

B_SEQ = 80
L = 512
H = 768
SAMPLES = 10
T = 16
VIEW_RANGE = 2
TEMP = 0.2
AVG_EPS = 1e-6
COS_EPS = 1e-8

N_CORES = 8
SPC = SAMPLES  # sequences per core = one dialogue
P = 128
LC = 384  # compacted token count (attention_mask=1 tokens only, zero-padded)
CH = LC // P  # 3 chunks
HB = H // P  # 6 h-blocks
TT = 2 * T  # 32
GSC = 2.0**-6  # keeps a*gam within fp8e4 range; cancels in cosine

# merged per-sequence input row layout (fp8 bytes). The G-pass inputs come
# first so the load can be split into two DMAs and compute starts after the
# first piece (hxt is only needed by the later U-pass).
OFF_HX = 0
OFF_ABX = CH * H  # 2304
OFF_AB2 = OFF_ABX + CH * TT  # 2400
OFF_ABD = OFF_AB2 + CH * TT  # 2496
OFF_HXT = OFF_ABD + CH * 4  # 2508
MROW = OFF_HXT + HB * LC  # 4812

_CACHE: dict = {}


def _build_nc():
    from contextlib import ExitStack

    import concourse.bacc as bacc
    import concourse.mybir as mybir
    import concourse.tile as tile

    f32 = mybir.dt.float32
    bf16 = mybir.dt.bfloat16
    f8 = mybir.dt.float8e4
    add = mybir.AluOpType.add

    nc = bacc.Bacc(
        "TRN2",
        debug=False,
        enable_asserts=False,
        target_bir_lowering=False,
    )

    mg = nc.dram_tensor("mg", [SPC, P, MROW], f8, kind="ExternalInput").ap()
    out = nc.dram_tensor("out", [SPC, 4, H], f32, kind="ExternalOutput").ap()

    with tile.TileContext(nc) as tc, ExitStack() as ctx:
        mgp = ctx.enter_context(tc.tile_pool(name="mgp", bufs=6))
        gtp = ctx.enter_context(tc.tile_pool(name="gtp", bufs=3))
        scp = ctx.enter_context(tc.tile_pool(name="scp", bufs=2))
        gmp = ctx.enter_context(tc.tile_pool(name="gmp", bufs=2))
        osp = ctx.enter_context(tc.tile_pool(name="osp", bufs=2))
        pgp = ctx.enter_context(tc.tile_pool(name="pgp", bufs=2, space="PSUM"))
        pup = ctx.enter_context(tc.tile_pool(name="pup", bufs=2, space="PSUM"))
        pdp = ctx.enter_context(tc.tile_pool(name="pdp", bufs=2, space="PSUM"))

        mgs, pus, gams = {}, {}, {}

        def load_p1(s):
            """hx + masks (everything the G- and D-passes read)."""
            mgt = mgp.tile([P, MROW], f8, name=f"mg{s}", tag="mg")
            mgs[s] = mgt
            if s == 0:
                # fine-grained first load: G(0)'s ci=0 matmuls start after
                # ~124KB instead of the full 321KB piece
                nc.sync.dma_start(
                    mgt[:, OFF_ABX:OFF_HXT], mg[s][:, OFF_ABX:OFF_HXT]
                )
                for ci in range(CH):
                    nc.sync.dma_start(
                        mgt[:, ci * H : (ci + 1) * H], mg[s][:, ci * H : (ci + 1) * H]
                    )
            else:
                nc.sync.dma_start(mgt[:, 0:OFF_HXT], mg[s][:, 0:OFF_HXT])

        def load_p2(s):
            """hxt (read only by the U-pass)."""
            mgt = mgs[s]
            nc.sync.dma_start(mgt[:, OFF_HXT:MROW], mg[s][:, OFF_HXT:MROW])

        def g_pass(s):
            mgt = mgs[s]
            pg = pgp.tile([P, HB * TT], f32, name=f"pg{s}", tag="pg")
            for hb in range(HB):
                for ci in range(CH):
                    nc.tensor.matmul(
                        pg[:, hb * TT : (hb + 1) * TT],
                        mgt[:, ci * H + hb * P : ci * H + (hb + 1) * P],
                        mgt[:, OFF_ABX + ci * TT : OFF_ABX + (ci + 1) * TT],
                        start=(ci == 0),
                        stop=(ci == CH - 1),
                    )
            gt = gtp.tile([P, HB * TT], f8, name=f"gt{s}", tag="gt")
            # split across both engines so U unblocks sooner
            half = HB * TT // 2
            nc.vector.tensor_copy(gt[:, 0:half], pg[:, 0:half])
            nc.scalar.copy(gt[:, half:], pg[:, half:])
            return gt

        def u_pass(s, gt):
            mgt = mgs[s]
            pu = pup.tile([P, CH * TT], f32, name=f"pu{s}", tag="pu")
            for ci in range(CH):
                for hb in range(HB):
                    nc.tensor.matmul(
                        pu[:, ci * TT : (ci + 1) * TT],
                        mgt[:, OFF_HXT + hb * LC + ci * P : OFF_HXT + hb * LC + (ci + 1) * P],
                        gt[:, hb * TT : (hb + 1) * TT],
                        start=(hb == 0),
                        stop=(hb == HB - 1),
                    )
            # gam = rowsum(U ∘ AB2) per chunk; tensor_tensor_reduce is broken
            # on this HW, so one DVE mul + per-chunk DVE reduces instead.
            # GSC is pre-baked into the host-side [a, b] columns of abd.
            scr = scp.tile([P, CH * TT], bf16, name=f"sc{s}", tag="sc")
            gam = gmp.tile([P, CH], f32, name=f"ga{s}", tag="ga")
            nc.vector.tensor_mul(scr[:], pu[:], mgt[:, OFF_AB2 : OFF_AB2 + CH * TT])
            # one strided reduce: [128, (3, 32)] -> [128, 3]
            nc.vector.tensor_reduce(
                gam[:],
                scr[:].rearrange("p (c t) -> p c t", c=CH),
                axis=mybir.AxisListType.X,
                op=add,
            )
            for ci in range(CH):
                # abd chunk cols [a*GSC, b*GSC, ., .] -> cols 2:4 = gam-scaled
                nc.vector.tensor_scalar_mul(
                    mgt[:, OFF_ABD + 4 * ci + 2 : OFF_ABD + 4 * ci + 4],
                    mgt[:, OFF_ABD + 4 * ci : OFF_ABD + 4 * ci + 2],
                    gam[:, ci : ci + 1],
                )

        def d_pack(s0, s1, pi):
            pd = pdp.tile([P, H], f32, name=f"pd{pi}", tag="pd")
            # adjacent matmuls target different column groups (partitions 0-3
            # vs 32-35), so the PE runs each pair concurrently
            for ci in range(CH):
                for n0, n1 in ((0, 512), (512, H)):
                    for j, s in ((0, s0), (1, s1)):
                        mgt = mgs[s]
                        nc.tensor.matmul(
                            pd[32 * j : 32 * j + 4, n0:n1],
                            mgt[:, OFF_ABD + 4 * ci : OFF_ABD + 4 * ci + 4],
                            mgt[:, ci * H + n0 : ci * H + n1],
                            start=(ci == 0),
                            stop=(ci == CH - 1),
                            skip_group_check=True,
                        )
            ot = osp.tile([36, H], f32, name=f"ot{pi}", tag="ot")
            if pi % 2 == 0:
                nc.scalar.copy(ot[0:4, :], pd[0:4, :])
                nc.scalar.copy(ot[32:36, :], pd[32:36, :])
            else:
                nc.vector.tensor_copy(ot[0:4, :], pd[0:4, :])
                nc.vector.tensor_copy(ot[32:36, :], pd[32:36, :])
            nc.sync.dma_start(out[s0], ot[0:4, :])
            nc.sync.dma_start(out[s1], ot[32:36, :])

        # software pipeline: load(s+2) | G(s) | U(s-1) | D(s-3, s-2).
        # D two iterations behind gam so its inputs are always ready and the
        # PE never waits on the DVE chain.
        gts = {}
        load_p1(0)
        load_p2(0)
        load_p1(1)
        for s in range(SPC + 3):
            if s < SPC:
                if s + 1 < SPC:
                    load_p2(s + 1)
                if s + 2 < SPC:
                    load_p1(s + 2)
                gts[s] = g_pass(s)
            if 1 <= s <= SPC:
                u_pass(s - 1, gts.pop(s - 1))
            if s >= 3 and s % 2 == 1:
                d_pack(s - 3, s - 2, (s - 3) // 2)

    nc.compile()
    return nc


def _prep_core_inputs(hidden_states, attention_mask, role_ids, turn_ids):
    """Per-core input maps: one-hot / band-smeared mask prep (index work only)."""
    import ml_dtypes

    f8 = ml_dtypes.float8_e4m3

    active = attention_mask != 0
    counts = active.sum(-1)
    assert counts.max() <= LC, f"active tokens {counts.max()} exceed LC={LC}"
    # stable-sort active tokens to the front; padded positions carry zero masks
    sel = np.argsort(~active, axis=1, kind="stable")[:, :LC]

    am = np.take_along_axis(active, sel, axis=1).astype(np.float32)
    ro = np.take_along_axis(role_ids, sel, axis=1)
    tu = np.take_along_axis(turn_ids, sel, axis=1)
    hc = np.take_along_axis(hidden_states, sel[..., None], axis=1)

    a = am * (ro == 0)
    b = am * (ro == 1)
    onehot = (tu[..., None] == np.arange(T, dtype=tu.dtype)).astype(np.float32)
    A1 = onehot * a[..., None]
    B1 = onehot * b[..., None]
    band = (
        np.abs(np.arange(T)[:, None] - np.arange(T)[None, :]) <= VIEW_RANGE
    ).astype(np.float32)
    # G = ABX^T h = [Band R_T ; Band Q_T]; gam selector AB2 = [A1 | B1]
    ABX = np.concatenate([B1 @ band, A1 @ band], axis=-1)  # [B, LC, 32]
    AB2 = np.concatenate([A1, B1], axis=-1)

    def chunked(x):
        f = x.shape[-1]
        return (
            x.reshape(B_SEQ, CH, P, f).transpose(0, 2, 1, 3).reshape(B_SEQ, P, CH * f)
        )

    hq = hc.astype(f8)  # quantize once; all views share the same values
    hx = chunked(hq.astype(np.float32)).astype(f8)  # [B, P, CH*H]
    # hxt[s, p, hb*LC + l] = h[s, l, hb*P + p]
    hxt = (
        np.ascontiguousarray(
            hq.astype(np.float32).reshape(B_SEQ, LC, HB, P).transpose(0, 3, 2, 1)
        )
        .reshape(B_SEQ, P, HB * LC)
        .astype(f8)
    )
    abx = chunked(ABX).astype(f8)
    ab2 = chunked(AB2).astype(f8)
    # [a, b] columns pre-scaled by GSC so the device-side gam products stay in
    # fp8 range; the uniform scale cancels in the cosine
    abd = np.zeros((B_SEQ, P, CH * 4), np.float32)
    ab_ch = chunked(np.stack([a, b], axis=-1)) * GSC  # [B, P, CH*2]
    for ci in range(CH):
        abd[:, :, 4 * ci : 4 * ci + 2] = ab_ch[:, :, 2 * ci : 2 * ci + 2]
    abd = abd.astype(f8)

    mgall = np.concatenate(
        [
            hx.view(np.uint8),
            abx.view(np.uint8),
            ab2.view(np.uint8),
            abd.view(np.uint8),
            hxt.view(np.uint8),
        ],
        axis=-1,
    ).view(f8)
    assert mgall.shape == (B_SEQ, P, MROW)

    in_maps = []
    for c in range(N_CORES):
        sl = slice(c * SPC, (c + 1) * SPC)
        in_maps.append({"mg": np.ascontiguousarray(mgall[sl])})

    # cheap integrity reference: qs/rs rows recomputed on host from the same fp8 h
    hq32 = hq.astype(np.float32)
    qs_ref = np.einsum("bl,blh->bh", a, hq32) * GSC
    rs_ref = np.einsum("bl,blh->bh", b, hq32) * GSC
    return in_maps, a.sum(-1), b.sum(-1), qs_ref, rs_ref


def _outputs_ok(outs, qs_ref, rs_ref):
    """Detect corrupted device runs: finite outputs + qs/rs rows match host."""
    vecs = np.concatenate(outs, axis=0).reshape(-1, 4, H)
    if not np.isfinite(vecs).all():
        return False
    for got, ref in ((vecs[:, 0], qs_ref), (vecs[:, 1], rs_ref)):
        num = np.linalg.norm(got - ref, axis=-1)
        den = np.linalg.norm(ref, axis=-1) + 1e-6
        if (num / den).max() > 0.05:
            return False
    return True


def _finalize(outs, labels, na, nb):
    """Host-side O(B*H) reduction: cosine, log-softmax, label-weighted loss.

    Rows per sequence: [qs, rs, qc*2^-6, rc*2^-6]; the scale and the
    mask-count denominators cancel inside the cosine.
    """
    vecs = np.concatenate(outs, axis=0).astype(np.float64).reshape(-1, 4, H)
    qs = vecs[:, 0] / (na + AVG_EPS)[:, None]
    rs = vecs[:, 1] / (nb + AVG_EPS)[:, None]
    qc = vecs[:, 2] / (nb + AVG_EPS)[:, None]
    rc = vecs[:, 3] / (na + AVG_EPS)[:, None]

    def cos(x, y):
        nx = np.maximum(np.linalg.norm(x, axis=-1), COS_EPS)
        ny = np.maximum(np.linalg.norm(y, axis=-1), COS_EPS)
        return (x * y).sum(-1) / (nx * ny)

    logit_q = (cos(qs, qc) / TEMP).reshape(-1, SAMPLES)
    logit_r = (cos(rs, rc) / TEMP).reshape(-1, SAMPLES)

    def lsm(x):
        m = x.max(-1, keepdims=True)
        e = np.exp(x - m)
        return x - m - np.log(e.sum(-1, keepdims=True))

    lab = labels.astype(np.float64)
    loss_q = -np.mean(lsm(logit_q) * lab)
    loss_r = -np.mean(lsm(logit_r) * lab)
    return np.float32(loss_r + loss_q)


def kernel(hidden_states, labels, attention_mask, role_ids, turn_ids):
    import time

    from concourse.bass_utils import run_bass_kernel_spmd

    if "nc" not in _CACHE:
        _CACHE["nc"] = _build_nc()
    nc = _CACHE["nc"]

    in_maps, na, nb, qs_ref, rs_ref = _prep_core_inputs(
        np.asarray(hidden_states),
        np.asarray(attention_mask),
        np.asarray(role_ids),
        np.asarray(turn_ids),
    )
    trace = bool(os.environ.get("BASS_KERNEL_TRACE"))

    # the axon/NRT path very occasionally drops a run; validate cheaply and retry
    outs = None
    for attempt in range(3):
        try:
            res = run_bass_kernel_spmd(
                nc, in_maps, core_ids=list(range(N_CORES)), trace=trace
            )
            cand = [res.results[c]["out"].reshape(SPC * 4, H) for c in range(N_CORES)]
        except Exception as e:
            import traceback

            print(f"[kernel] attempt {attempt} failed: {type(e).__name__}: {e}")
            traceback.print_exc()
            if attempt == 2:
                raise
            time.sleep(2.0)
            continue
        outs = cand
        if _outputs_ok(cand, qs_ref, rs_ref):
            break
    if trace:
        _CACHE["last_results"] = res
        print(
            f"[kernel] exec_time_ns={res.exec_time_ns} "
            f"mean_exec_time_ns={res.mean_exec_time_ns}"
        )
    return _finalize(outs, np.asarray(labels), na, nb)
